# revision 6
# baseline (speedup 1.0000x reference)
"""Memory-efficient multi-head attention on 8 Trainium2 NeuronCores.

Problem (hardcoded): B=2, Nq=Nk=2048, C=512, H=8 heads, D=64.
  out = softmax((x_q Wq^T + bq)(x_k Wk^T + bk)^T / sqrt(D) + mask) (x_v Wv^T + bv) Wo^T + bo

Sharding: core c in 0..7 handles batch b = c//4 and head pair (2*(c%4), 2*(c%4)+1).
Each core computes its head pair's full attention and the partial output
projection (Wo columns for its heads); host sums the 4 partials per batch.

Device algorithm (per core), all activations kept transposed ([C, N] layouts):
  Q^T = Wq_p x_q^T   [128(hd pair), 2048]     (x^T pre-transposed on host)
  K^T = Wk_p x_k^T   [128, 2048]
  V   = x_v^T-sliced matmuls -> natural [k, d] tiles augmented with a ones
        column: lhsT [128, 65] so P V also yields sum(exp) for free.
  Per q-block (512) and k-tile (128):
    S^T(h1|h2) via two row-packed concurrent matmuls (K=64 each, fp32r)
    exp on ACT: es = Exp(0.125 * S^T + mask_bias)   [128, 1024] one instr
    P V: po_h += [V_h | 1]^T es_h                   (M=65, accumulates over kt)
  Normalize: recip = 1/po[64]; broadcast via ones-matmul; o_h = po[0:64] * recipB
  Out: out^T[ct] = Wo_p[:, ct]^T @ [o_h1; o_h2] + bo  -> DRAM (512, 2048)
"""
import sys

for _p in ("/opt/trn_rl_repo",):
    if _p not in sys.path:
        sys.path.append(_p)

from contextlib import ExitStack

import numpy as np

import concourse.bass as bass
import concourse.tile as tile
from concourse import bacc, mybir
from concourse import bass_utils

F = mybir.dt.float32
R = mybir.dt.float32r
EXPF = mybir.ActivationFunctionType.Exp

B, N, C, H, D = 2, 2048, 512, 8, 64
SCALE = D ** -0.5
CI = C // 128   # 4 c_in tiles
KT = N // 128   # 16 k tiles
QT = N // 512   # 4 q blocks
NEG = -30000.0  # mask bias: exp(s*scale + NEG) == 0 in fp32

_NC_CACHE = {}


def _build():
    nc = bacc.Bacc("TRN2", target_bir_lowering=False, debug=False)
    d = {}
    for name, shape in [
        ("xqT", (C, N)), ("xkT", (C, N)), ("xvT", (C, N)),
        ("wq", (128, CI, 128)), ("wk", (128, CI, 128)), ("wv", (128, CI, 128)),
        ("wo1", (64, C)), ("wo2", (64, C)),
        ("bqp", (128, 1)), ("bkp", (128, 1)), ("bvp", (1, 128)),
        ("bop", (128, QT)), ("mb", (128, KT)),
    ]:
        d[name] = nc.dram_tensor(name, shape, F, kind="ExternalInput").ap()
    outT = nc.dram_tensor("outT", (C, N), F, kind="ExternalOutput").ap()

    with ExitStack() as ctx:
        tc = ctx.enter_context(tile.TileContext(nc))
        wpool = ctx.enter_context(tc.tile_pool(name="w", bufs=1))
        xpool = ctx.enter_context(tc.tile_pool(name="x", bufs=6))
        xrpool = ctx.enter_context(tc.tile_pool(name="xr", bufs=6))
        apool = ctx.enter_context(tc.tile_pool(name="act", bufs=1))
        espool = ctx.enter_context(tc.tile_pool(name="es", bufs=3))
        onpool = ctx.enter_context(tc.tile_pool(name="on", bufs=4))
        outp = ctx.enter_context(tc.tile_pool(name="out", bufs=3))
        rpool = ctx.enter_context(tc.tile_pool(name="r", bufs=4))
        pproj = ctx.enter_context(tc.tile_pool(name="pproj", bufs=2, space="PSUM"))
        pps = ctx.enter_context(tc.tile_pool(name="pps", bufs=2, space="PSUM"))
        ppo = ctx.enter_context(tc.tile_pool(name="ppo", bufs=2, space="PSUM"))

        # ---- constants / weights ----
        def loadw(name, shape):
            raw = wpool.tile(list(shape), F, tag=name + "_raw")
            nc.sync.dma_start(out=raw, in_=d[name])
            rt = wpool.tile(list(shape), R, tag=name)
            nc.gpsimd.tensor_copy(out=rt, in_=raw)
            return rt

        wq = loadw("wq", (128, CI, 128))
        wk = loadw("wk", (128, CI, 128))
        wv = loadw("wv", (128, CI, 128))
        wo1 = loadw("wo1", (64, C))
        wo2 = loadw("wo2", (64, C))
        bvp = loadw("bvp", (1, 128))

        onesf = wpool.tile([1, 128], F, tag="onesf")
        nc.vector.memset(onesf, 1.0)
        ones = wpool.tile([1, 128], R, tag="ones")
        nc.gpsimd.tensor_copy(out=ones, in_=onesf)
        onescol_f = wpool.tile([128, 1], F, tag="onescol_f")
        nc.vector.memset(onescol_f, 1.0)

        bqp = wpool.tile([128, 1], F, tag="bqp")
        nc.sync.dma_start(out=bqp, in_=d["bqp"])
        bkp = wpool.tile([128, 1], F, tag="bkp")
        nc.sync.dma_start(out=bkp, in_=d["bkp"])
        bop = wpool.tile([128, QT], F, tag="bop")
        nc.sync.dma_start(out=bop, in_=d["bop"])
        mb = wpool.tile([128, KT], F, tag="mb")
        nc.sync.dma_start(out=mb, in_=d["mb"])

        # persistent activation tiles
        q_sb = apool.tile([128, N], R, tag="q_sb")
        k_sb = apool.tile([128, N], R, tag="k_sb")
        v_t = []
        for kt in range(KT):
            vt = apool.tile([128, 130], R, tag=f"v{kt}")
            # ones columns (64 and 129), rounded to f32r via gpsimd
            nc.gpsimd.tensor_copy(out=vt[:, 64:65], in_=onescol_f)
            nc.gpsimd.tensor_copy(out=vt[:, 129:130], in_=onescol_f)
            v_t.append(vt)

        # ---- projections: Q and K streamed in 512-wide column groups ----
        def proj_group(xname, g, wt, bias, dst):
            chunks = []
            for ci in range(CI):
                raw = xpool.tile([128, 512], F, tag="x")
                nc.sync.dma_start(
                    out=raw, in_=d[xname][ci * 128:(ci + 1) * 128, g * 512:(g + 1) * 512]
                )
                ch = xrpool.tile([128, 512], R, tag="xr")
                nc.vector.tensor_copy(out=ch, in_=raw)  # fp32 2x SBUF copy + round
                chunks.append(ch)
            pq = pproj.tile([128, 512], F, tag="proj")
            for ci in range(CI):
                nc.tensor.matmul(pq, lhsT=wt[:, ci, :], rhs=chunks[ci],
                                 start=(ci == 0), stop=(ci == CI - 1))
            nc.vector.tensor_scalar_add(dst[:, g * 512:(g + 1) * 512], pq, bias)

        for g in range(QT):
            proj_group("xqT", g, wq, bqp, q_sb)
            proj_group("xkT", g, wk, bkp, k_sb)

        # ---- V projection (natural layout, per k-tile) ----
        for g in range(QT):
            chunks = []
            for ci in range(CI):
                raw = xpool.tile([128, 512], F, tag="x")
                nc.sync.dma_start(
                    out=raw, in_=d["xvT"][ci * 128:(ci + 1) * 128, g * 512:(g + 1) * 512]
                )
                ch = xrpool.tile([128, 512], R, tag="xr")
                nc.vector.tensor_copy(out=ch, in_=raw)
                chunks.append(ch)
            for j in range(4):
                kt = 4 * g + j
                pv = pproj.tile([128, 128], F, tag="proj")
                for ci in range(CI):
                    nc.tensor.matmul(pv, lhsT=chunks[ci][:, j * 128:(j + 1) * 128],
                                     rhs=wv[:, ci, :], start=(ci == 0), stop=False)
                nc.tensor.matmul(pv, lhsT=ones, rhs=bvp, start=False, stop=True)
                nc.vector.tensor_copy(out=v_t[kt][:, 0:64], in_=pv[:, 0:64])
                nc.vector.tensor_copy(out=v_t[kt][:, 65:129], in_=pv[:, 64:128])

        # ---- attention ----
        for qt in range(QT):
            qs = slice(qt * 512, (qt + 1) * 512)
            po1 = ppo.tile([65, 512], F, tag="po")
            po2 = ppo.tile([65, 512], F, tag="po")
            for kt in range(KT):
                ks = slice(kt * 128, (kt + 1) * 128)
                ps = pps.tile([128, 1024], F, tag="ps")
                nc.tensor.matmul(ps[:, 0:512], lhsT=k_sb[0:64, ks], rhs=q_sb[0:64, qs],
                                 start=True, stop=True)
                nc.tensor.matmul(ps[:, 512:1024], lhsT=k_sb[64:128, ks], rhs=q_sb[64:128, qs],
                                 start=True, stop=True)
                es = espool.tile([128, 1024], R, tag="es")
                nc.scalar.activation(out=es, in_=ps, func=EXPF,
                                     bias=mb[:, kt:kt + 1], scale=SCALE)
                nc.tensor.matmul(po1, lhsT=v_t[kt][:, 0:65], rhs=es[:, 0:512],
                                 start=(kt == 0), stop=(kt == KT - 1))
                nc.tensor.matmul(po2, lhsT=v_t[kt][:, 65:130], rhs=es[:, 512:1024],
                                 start=(kt == 0), stop=(kt == KT - 1))
            ons = []
            for po in (po1, po2):
                rc = rpool.tile([1, 512], R, tag="rc")
                with nc.allow_low_precision(reason="f32r is 4-byte; rounding only"):
                    nc.vector.reciprocal(rc, po[64:65, :])
                pr = pproj.tile([65, 512], F, tag="proj")
                nc.tensor.matmul(pr, lhsT=ones[0:1, 0:65], rhs=rc, start=True, stop=True)
                prs = rpool.tile([64, 512], F, tag="prs")
                nc.vector.tensor_copy(prs, pr[0:64, :])
                on = onpool.tile([64, 512], R, tag="on")
                nc.vector.tensor_mul(on, po[0:64, :], prs)
                ons.append(on)
            for ct in range(CI):
                cs = slice(ct * 128, (ct + 1) * 128)
                pz = pproj.tile([128, 512], F, tag="proj")
                nc.tensor.matmul(pz, lhsT=wo1[:, cs], rhs=ons[0], start=True, stop=False)
                nc.tensor.matmul(pz, lhsT=wo2[:, cs], rhs=ons[1], start=False, stop=True)
                ot = outp.tile([128, 512], F, tag="ot")
                nc.vector.tensor_scalar_add(ot, pz, bop[:, ct:ct + 1])
                nc.sync.dma_start(out=outT[cs, qs], in_=ot)

    nc.compile()
    return nc


def get_nc():
    if "nc" not in _NC_CACHE:
        _NC_CACHE["nc"] = _build()
    return _NC_CACHE["nc"]


def shard_inputs(query, key, value, key_padding_mask, Wq, bq, Wk, bk, Wv, bv, Wo, bo):
    """Full inputs -> list of 8 per-core input dicts (host-side layout prep only)."""
    in_maps = []
    f32 = np.float32
    for c in range(8):
        b, hp = c // 4, c % 4
        rows = slice(hp * 128, (hp + 1) * 128)

        def wtile(W):
            # (512, 128) slice of W.T -> (128, CI, 128): [p, ci, j] = W.T[ci*128+p, j]
            t = np.ascontiguousarray(W[rows, :].T.astype(f32))
            return np.ascontiguousarray(t.reshape(CI, 128, 128).transpose(1, 0, 2))

        wo_p = Wo[:, rows].T.astype(f32)  # (128, C): rows = head-pair dims
        mbv = np.where(key_padding_mask[b], f32(NEG), f32(0.0)).astype(f32)
        in_maps.append({
            "xqT": np.ascontiguousarray(query[b].T.astype(f32)),
            "xkT": np.ascontiguousarray(key[b].T.astype(f32)),
            "xvT": np.ascontiguousarray(value[b].T.astype(f32)),
            "wq": wtile(Wq), "wk": wtile(Wk), "wv": wtile(Wv),
            "wo1": np.ascontiguousarray(wo_p[0:64, :]),
            "wo2": np.ascontiguousarray(wo_p[64:128, :]),
            "bqp": np.ascontiguousarray(bq[rows].astype(f32).reshape(128, 1)),
            "bkp": np.ascontiguousarray(bk[rows].astype(f32).reshape(128, 1)),
            "bvp": np.ascontiguousarray(bv[rows].astype(f32).reshape(1, 128)),
            "bop": np.ascontiguousarray(
                (bo.astype(f32) if hp == 0 else np.zeros(C, f32)).reshape(QT, 128).T
            ),
            "mb": np.ascontiguousarray(mbv.reshape(KT, 128).T),
        })
    return in_maps


def unshard_outputs(results):
    out = np.empty((B, N, C), np.float32)
    for b in range(B):
        acc = results[4 * b]["outT"].astype(np.float32).copy()
        for i in range(1, 4):
            acc += results[4 * b + i]["outT"]
        out[b] = acc.T
    return out


def kernel(**inputs):
    nc = get_nc()
    in_maps = shard_inputs(**{k: np.asarray(v) for k, v in inputs.items()})
    res = bass_utils.run_bass_kernel_spmd(nc, in_maps, core_ids=list(range(8)))
    return unshard_outputs(res.results)
